# revision 3
# baseline (speedup 1.0000x reference)
"""BigBird sparse attention kernel for Trainium2 (8 NeuronCores).

Problem (hardcoded): B=2, S=2048, H=16, D=64, block=128, G=128 global
tokens, R=64 random tokens, attn_mask is all-zeros by construction
(spec fill="zeros").

Math notes (mask == 0):
  * Diagonal branch: standard per-(b, block, head) softmax attention
    within each 128-token diagonal block.
  * Global branch: the reference contracts softmax weights only over
    their own row (einsum 'bhgs,bghd->bghd'), so the contribution is
    v[:, :G] * rowsum(softmax) == v[:, :G] (rowsum == 1 up to fp
    rounding).
  * Random branch: same structure ('bhnm,bmhd->bnhd' with size-1
    broadcast), contribution is v[:, r] * rowsum(softmax) == v[:, r],
    scatter-added per occurrence of each random index.
  Both reduce to out[:, s] += cnt2[s] * v[:, s] with
  cnt2 = bincount(rand_indices) + (s < G).

Sharding: each of the 8 cores gets one (batch, 4-head group):
core c -> b = c // 4, heads 4*(c%4) .. 4*(c%4)+4. All branches are
independent per (b, h); no collectives.

Layout trick: q/k are pre-transposed on host to (d, s) per head so the
on-chip matmuls need no transposes at all:
  mm1: S^T[k,q] = sum_d K^T[d,k] * Q^T[d,q]   (lhsT=K^T, rhs=Q^T, K=64
       contraction -> two heads packed in the PE via 64-row tiles)
  mm2: O[q,:]   = sum_k exp(S^T)[k,q] * [V|1][k,:]  (lhsT=exp tile
       directly, no transpose; extra ones column yields the softmax
       denominator for free)
"""

import os

import numpy as np

B, S, H, D = 2, 2048, 16, 64
BS = 128          # block size
NB = S // BS      # 16 diagonal blocks
G = 128           # num global tokens
SCALE = 1.0 / float(D) ** 0.5
NCORES = 8
HPC = 4           # heads per core
SB2 = 2           # s-blocks per DMA super-block

_cached = {}


def _build_program():
    import concourse.bass as bass
    import concourse.tile as tile
    from concourse import bacc, mybir

    f32 = mybir.dt.float32
    AF = mybir.ActivationFunctionType
    ALU = mybir.AluOpType

    nc = bacc.Bacc(
        "TRN2",
        target_bir_lowering=False,
        debug=False,
        enable_asserts=False,
        num_devices=NCORES,
    )
    # rows of qT/kT: head-pair i holds heads (2i, 2i+1) as partition
    # p = (h % 2) * 64 + d; cols are the full sequence.
    qT = nc.dram_tensor("qT", [2, 128, S], f32, kind="ExternalInput").ap()
    kT = nc.dram_tensor("kT", [2, 128, S], f32, kind="ExternalInput").ap()
    v = nc.dram_tensor("v", [S, HPC, D], f32, kind="ExternalInput").ap()
    c2 = nc.dram_tensor("c2", [128, NB], f32, kind="ExternalInput").ap()
    out = nc.dram_tensor("out", [S, HPC, D], f32, kind="ExternalOutput").ap()

    with tile.TileContext(nc) as tc:
        with (
            tc.tile_pool(name="const", bufs=1) as cpool,
            tc.tile_pool(name="qk", bufs=3) as qkpool,
            tc.tile_pool(name="vp", bufs=3) as vpool,
            tc.tile_pool(name="wp", bufs=6) as wpool,
            tc.tile_pool(name="outp", bufs=3) as opool,
            tc.tile_pool(name="rp", bufs=8) as rpool,
            tc.tile_pool(name="stps", bufs=4, space="PSUM") as stpool,
            tc.tile_pool(name="ops", bufs=4, space="PSUM") as oppool,
        ):
            c2_t = cpool.tile([128, NB], f32)
            nc.sync.dma_start(c2_t[:], c2[:])

            for sb2 in range(NB // SB2):
                sl2 = slice(sb2 * SB2 * BS, (sb2 + 1) * SB2 * BS)
                # 2 s-blocks x (2 head-pairs) of q^T and k^T
                qt0 = qkpool.tile([128, SB2 * BS], f32, tag="qt0")
                nc.sync.dma_start(qt0[:], qT[0, :, sl2])
                qt1 = qkpool.tile([128, SB2 * BS], f32, tag="qt1")
                nc.sync.dma_start(qt1[:], qT[1, :, sl2])
                kt0 = qkpool.tile([128, SB2 * BS], f32, tag="kt0")
                nc.sync.dma_start(kt0[:], kT[0, :, sl2])
                kt1 = qkpool.tile([128, SB2 * BS], f32, tag="kt1")
                nc.sync.dma_start(kt1[:], kT[1, :, sl2])

                for sbl in range(SB2):
                    sb = sb2 * SB2 + sbl
                    sl = slice(sb * BS, (sb + 1) * BS)
                    ssl = slice(sbl * BS, (sbl + 1) * BS)

                    v_t = vpool.tile([128, HPC, D + 1], f32, tag="v")
                    nc.sync.dma_start(v_t[:, :, 0:D], v[sl])
                    nc.gpsimd.memset(v_t[:, :, D : D + 1], 1.0)
                    # vs = cnt2[s] * v  (ScalarE copy with per-partition scale)
                    vs_t = vpool.tile([128, HPC, D], f32, tag="vs")
                    nc.scalar.mul(vs_t[:], v_t[:, :, 0:D], c2_t[:, sb : sb + 1])

                    sts = []
                    for h in range(HPC):
                        pair, sub = divmod(h, 2)
                        qt = (qt0, qt1)[pair]
                        kt = (kt0, kt1)[pair]
                        dsl = slice(sub * 64, (sub + 1) * 64)
                        st = stpool.tile([128, BS], f32, tag="st")
                        nc.tensor.matmul(
                            st[:],
                            lhsT=kt[dsl, ssl],
                            rhs=qt[dsl, ssl],
                            start=True,
                            stop=True,
                        )
                        sts.append(st)

                    ws = []
                    for h in range(HPC):
                        w_t = wpool.tile([128, BS], f32, tag="w")
                        nc.scalar.activation(w_t[:], sts[h][:], AF.Exp, scale=SCALE)
                        ws.append(w_t)

                    os_ = []
                    for h in range(HPC):
                        o_ps = oppool.tile([128, D + 1], f32, tag="o")
                        nc.tensor.matmul(
                            o_ps[:], lhsT=ws[h][:], rhs=v_t[:, h, :],
                            start=True, stop=True,
                        )
                        os_.append(o_ps)

                    out_t = opool.tile([128, HPC, D], f32, tag="out")
                    for h in range(HPC):
                        rec = rpool.tile([128, 1], f32, tag="rec")
                        nc.vector.reciprocal(rec[:], os_[h][:, D : D + 1])
                        # out = o_unnorm * (1/rowsum) + cnt2 * v
                        nc.vector.scalar_tensor_tensor(
                            out_t[:, h, :],
                            os_[h][:, 0:D],
                            rec[:],
                            vs_t[:, h, :],
                            ALU.mult,
                            ALU.add,
                        )
                    nc.sync.dma_start(out[sl], out_t[:])
    nc.compile()
    return nc


def _get_nc():
    if "nc" not in _cached:
        _cached["nc"] = _build_program()
    return _cached["nc"]


def _make_in_maps(q, k, v, rand_indices):
    q = np.asarray(q, dtype=np.float32)
    k = np.asarray(k, dtype=np.float32)
    v = np.asarray(v, dtype=np.float32)
    ri = np.asarray(rand_indices).astype(np.int64).ravel()

    cnt = np.bincount(ri, minlength=S).astype(np.float32)
    cnt[:G] += 1.0
    c2 = np.ascontiguousarray(cnt.reshape(NB, BS).T)  # (128, 16)

    in_maps = []
    for c in range(NCORES):
        b, hg = divmod(c, 4)
        hsl = slice(HPC * hg, HPC * (hg + 1))
        # (S, HPC, D) -> (HPC, D, S) -> (2 pairs, 128, S)
        qTc = np.ascontiguousarray(
            q[b, :, hsl, :].transpose(1, 2, 0)
        ).reshape(2, 128, S)
        kTc = np.ascontiguousarray(
            k[b, :, hsl, :].transpose(1, 2, 0)
        ).reshape(2, 128, S)
        vc = np.ascontiguousarray(v[b, :, hsl, :])  # (S, HPC, D)
        in_maps.append({"qT": qTc, "kT": kTc, "v": vc, "c2": c2})
    return in_maps


def _assemble(results):
    out = np.empty((B, S, H, D), dtype=np.float32)
    for c in range(NCORES):
        b, hg = divmod(c, 4)
        out[b, :, HPC * hg : HPC * (hg + 1), :] = results[c]["out"]
    return out


def _run(q, k, v, attn_mask, rand_indices, trace=False, trace_kwargs=None):
    from concourse.bass_utils import run_bass_kernel_spmd

    nc = _get_nc()
    in_maps = _make_in_maps(q, k, v, rand_indices)
    res = run_bass_kernel_spmd(
        nc,
        in_maps,
        list(range(NCORES)),
        trace=trace,
        **(trace_kwargs or {}),
    )
    return _assemble(res.results), res


def _reference_fallback(q, k, v, attn_mask, rand_indices):
    """Numpy replica of the reference for the (never expected per spec)
    case of a non-zero attn_mask."""
    q = np.asarray(q, np.float32)
    k = np.asarray(k, np.float32)
    v = np.asarray(v, np.float32)
    m = np.asarray(attn_mask, np.float32)
    ri = np.asarray(rand_indices).astype(np.int64).ravel()

    def softmax(x):
        x = x - x.max(axis=-1, keepdims=True)
        e = np.exp(x)
        return e / e.sum(axis=-1, keepdims=True)

    qb = q.reshape(B, NB, BS, H, D)
    kb = k.reshape(B, NB, BS, H, D)
    vb = v.reshape(B, NB, BS, H, D)
    scores = np.einsum("bnqhd,bnkhd->bnhqk", qb, kb) * SCALE
    mb = m.reshape(B, H, NB, BS, NB, BS)
    idx = np.arange(NB)
    diag = mb[:, :, idx, :, idx, :]  # (NB,B,H,BS,BS)
    scores = scores + diag.transpose(1, 0, 2, 3, 4)
    w = softmax(scores)
    out = np.einsum("bnhqk,bnkhd->bnqhd", w, vb).reshape(B, S, H, D)

    gq = q[:, :G]
    gv = v[:, :G]
    gs = np.einsum("bghd,bshd->bhgs", gq, k) * SCALE + m[:, :, :G, :]
    gw = softmax(gs)
    out[:, :G] += gv * gw.sum(axis=-1).transpose(0, 2, 1)[..., None]

    rq = q[:, ri]
    rv = v[:, ri]
    rs = np.einsum("brhd,bshd->bhrs", rq, k) * SCALE + m[:, :, ri, :]
    rw = softmax(rs)
    rowsum = rw.sum(axis=-1).transpose(0, 2, 1)  # (B,R,H)
    contrib = rv * rowsum[..., None]
    np.add.at(out, (slice(None), ri), contrib)
    return out


def kernel(q, k, v, attn_mask, rand_indices):
    am = np.asarray(attn_mask)
    if am.any():
        return _reference_fallback(q, k, v, attn_mask, rand_indices)
    out, _ = _run(q, k, v, attn_mask, rand_indices, trace=False)
    return out
